# revision 26
# baseline (speedup 1.0000x reference)
"""Depth-gated 3x3 conv (DepConv3D) Trainium2 Bass kernel.

Shapes (hardcoded): features (4,16,512,512) f32, depth (4,512,512) int32,
weight (32,16,3,3,3) f32 -> out (4,32,512,512) f32.

Strategy: 8-way data parallel over (batch, row-half). Each core computes a
(32, 256, 512) output slab.

Math: for output pixel p and tap k (3x3 neighborhood), the weight depth-slice
is selected by diff = depth[nb_k(p)] - depth[p]: diff==0 -> W[:,:,1,k],
diff==-1 -> W[:,:,0,k], else no contribution. The center tap always uses
W[:,:,1,center]. With bmask = (diff==0)-(diff==-1) in {0,+1,-1} and
amask = bmask^2, the gated weight is amask*(W1+W0)/2 + bmask*(W1-W0)/2.

REPL=1 (default) pipeline, per R-row iteration (NF pixels):
  - HBM DMA: compact x slab xs (16ch, (R+2) rows, flat W=512, no W pad)
    + bmask (128, NF) fp8.  Only ~0.9 MB/iter of HBM traffic.
  - 2 SBUF->SBUF DMAs replicate the slab into x_rep (128 = 8 taps x 16ch,
    NF): one 2x3-window DMA for the dh=+-1 taps (96 partitions), one 2-
    window DMA for the dh=0 taps (32 partitions). Tap windows are FLAT
    (wrap across rows); the wrong-but-masked values at w=0/511 are killed
    by host-zeroed bmask columns (reference contributes 0 there via its
    zero feature padding).
  - ACT: convert bmask fp8 -> bf16.
  - DVE: pB = bmask*x_rep, pA = bmask*pB (2 tensor_tensor ops, 2x mode).
  - PE per psum tile (4 col-tiled groups, tile_position=(0,32g)):
    psum = wB.T@pB + wC.T@xs_window + wA.T@pA   (center read straight
    from the slab, 16-partition contraction).
  - ACT evicts psum tiles -> (128, NT*W) bf16 staging, one DMA to HBM.
Emission is software-pipelined: dma(it) / replicate+convert(it-1) /
compute(it-2).

Partition layout (ch-major, imposed by the replicate-DMA iteration order):
  p in [0,32):   p = 2*ch + tap_b,  tap_b in [(0,-1), (0,1)]
  p in [32,128): p-32 = 6*ch + tap_a, tap_a in [(-1,-1),(-1,0),(-1,1),
                                                (1,-1),(1,0),(1,1)]
wA/wB/bmask use the same layout (host-side bookkeeping only).
"""

import sys
import threading

sys.path.insert(0, "/opt/trn_rl_repo")

import os
import numpy as np
import ml_dtypes

REPL = os.environ.get("REPL", "1") == "1"
REPL_Q = os.environ.get("REPL_Q", "sync")  # queue for replicate DMAs
PROBE_NO_C = os.environ.get("PROBE_NO_C", "0") == "1"  # timing-only probe

bf16 = ml_dtypes.bfloat16

B, iC, H, W = 4, 16, 512, 512
oC = 32
NCORES = 8
HC = H // 2  # rows per core (256)
R = 8        # rows per iteration
NT = R // 4  # psum tiles per iteration
NF = R * W   # free elements per iteration
N_ITERS = HC // R
SLEN = (R + 2) * W + 2   # slab tile elems per iteration
PADW = 64    # host pad elems so windowed reads stay in-bounds

TAP_B = [(0, -1), (0, 1)]
TAP_Am = [(-1, -1), (-1, 0), (-1, 1)]
TAP_Ap = [(1, -1), (1, 0), (1, 1)]


def _part_layout():
    """(tap, ch) per partition for the REPL layout (ch-major within each
    replicate-DMA's destination range, imposed by AP iteration order)."""
    out = [None] * 128
    for p in range(32):
        out[p] = (TAP_B[p % 2], p // 2)
    for q in range(48):
        out[32 + q] = (TAP_Am[q % 3], q // 3)
    for q in range(48):
        out[80 + q] = (TAP_Ap[q % 3], q // 3)
    return out


_prog_lock = threading.Lock()
_progs = {}


def _win_ap(base_ap, dims, offset_elems):
    """Hand-build an AP: dims = [(stride, size), ...] over base tensor."""
    ap = base_ap.copy()
    while ap.ndim > 1:
        ap = ap.flatten()
    ap = ap[offset_elems:offset_elems + 1]
    for _ in range(len(dims) - 1):
        ap = ap.unsqueeze(0)
    a = ap.ap
    for i, (st, sz) in enumerate(dims):
        a[i] = [st, sz]
    return ap


def _build_program(reps=1):
    import concourse.tile as tile
    from concourse import bacc, mybir
    from contextlib import ExitStack, nullcontext

    nc = bacc.Bacc("TRN2", target_bir_lowering=False, debug=False,
                   num_devices=NCORES)
    xs_d = nc.dram_tensor("xs", [iC, (HC + 2) * W + PADW], mybir.dt.bfloat16,
                          kind="ExternalInput").ap()
    bm_d = nc.dram_tensor("bm", [128, HC * W], mybir.dt.float8e4,
                          kind="ExternalInput").ap()
    wA = nc.dram_tensor("wA", [128, oC], mybir.dt.bfloat16,
                        kind="ExternalInput").ap()
    wB = nc.dram_tensor("wB", [128, oC], mybir.dt.bfloat16,
                        kind="ExternalInput").ap()
    wC = nc.dram_tensor("wC", [iC, oC], mybir.dt.bfloat16,
                        kind="ExternalInput").ap()
    y = nc.dram_tensor("y", [HC // R, 4, oC, NT, W], mybir.dt.bfloat16,
                       kind="ExternalOutput").ap()

    with tile.TileContext(nc) as tc:
        with ExitStack() as ctx:
            wpool = ctx.enter_context(tc.tile_pool(name="w", bufs=1))
            spool = ctx.enter_context(tc.tile_pool(name="s", bufs=4))
            inpool = ctx.enter_context(tc.tile_pool(name="in", bufs=3))
            xrpool = ctx.enter_context(tc.tile_pool(name="xr", bufs=3))
            mpool = ctx.enter_context(tc.tile_pool(name="m", bufs=3))
            opool = ctx.enter_context(tc.tile_pool(name="o", bufs=3))
            pspool = ctx.enter_context(
                tc.tile_pool(name="ps", bufs=8, space="PSUM"))

            wA_t = wpool.tile([128, oC], mybir.dt.bfloat16, tag="wA")
            wB_t = wpool.tile([128, oC], mybir.dt.bfloat16, tag="wB")
            wC_t = wpool.tile([iC, oC], mybir.dt.bfloat16, tag="wC")
            nc.sync.dma_start(wA_t[:], wA[:])
            nc.sync.dma_start(wB_t[:], wB[:])
            nc.sync.dma_start(wC_t[:], wC[:])

            def dma_in(it):
                h0 = it * R
                xs_t = spool.tile([iC, SLEN], mybir.dt.bfloat16, tag="xs")
                src = _win_ap(xs_d, [((HC + 2) * W + PADW, iC), (1, SLEN)],
                              h0 * W)
                nc.sync.dma_start(xs_t[:], src)
                bm8 = inpool.tile([128, NF], mybir.dt.float8e4, tag="bm8")
                bsrc = _win_ap(bm_d, [(HC * W, 128), (1, NF)], h0 * W)
                nc.sync.dma_start(bm8[:], bsrc)
                return xs_t, bm8

            def replicate_convert(st):
                xs_t, bm8 = st
                x_rep = xrpool.tile([128, NF], mybir.dt.bfloat16, tag="xr")
                # dh=0 taps -> partitions 0..31 (ch-major, tap-minor).
                # src iteration (ch, dw-window, free) matches dst
                # (partition, free) flattening. window base k0=513+dw.
                srcB = xs_t[0:iC, 512:512 + 1]
                srcB.ap[1] = [2, 2]    # dw window: k0 = 512, 514
                srcB = srcB.unsqueeze(2)
                srcB.ap[2] = [1, NF]   # free
                rq = getattr(nc, REPL_Q)
                rq.dma_start(x_rep[0:32, :], srcB)
                # dh=-1 taps -> partitions 32..79, dh=+1 -> 80..127
                for dhi, base in ((0, 0), (1, 2 * W)):
                    srcA = xs_t[0:iC, base:base + 1]
                    srcA.ap[1] = [1, 3]     # dw in {-1,0,1}
                    srcA = srcA.unsqueeze(2)
                    srcA.ap[2] = [1, NF]    # free
                    rq.dma_start(
                        x_rep[32 + 48 * dhi:80 + 48 * dhi, :], srcA)

                bm = mpool.tile([128, NF], mybir.dt.bfloat16, tag="bm")
                nc.scalar.copy(bm[:], bm8[:])
                return xs_t, x_rep, bm

            def compute(st, it):
                xs_t, x_rep, bm = st
                pB = mpool.tile([128, NF], mybir.dt.bfloat16, tag="pB")
                pA = mpool.tile([128, NF], mybir.dt.bfloat16, tag="pA")
                nc.vector.tensor_tensor(pB[:], bm[:], x_rep[:],
                                        mybir.AluOpType.mult)
                nc.vector.tensor_tensor(pA[:], bm[:], pB[:],
                                        mybir.AluOpType.mult)

                out_sb = opool.tile([128, NT * W], mybir.dt.bfloat16,
                                    tag="osb")
                for t in range(NT):
                    psum = pspool.tile([128, W], mybir.dt.float32,
                                       tag="psum")
                    passes = ((wB_t, pB, 128, 0, True, False),
                              (wC_t, xs_t, iC, 513, False, False),
                              (wA_t, pA, 128, 0, False, True))
                    if PROBE_NO_C:
                        passes = ((wB_t, pB, 128, 0, True, False),
                                  (wA_t, pA, 128, 0, False, True))
                    for lhsT, rhs, np_, off, start, stop in passes:
                        for g in range(4):
                            r = 4 * t + g
                            sl = slice(r * W + off, (r + 1) * W + off)
                            nc.tensor.matmul(psum[32 * g:32 * g + 32, :],
                                             lhsT[:], rhs[0:np_, sl],
                                             start=start, stop=stop,
                                             tile_position=(0, 32 * g),
                                             skip_group_check=True)
                    nc.scalar.copy(out_sb[:, t * W:(t + 1) * W], psum[:])

                ydst = _win_ap(y, [(NT * W, 128), (1, NT * W)],
                               it * 128 * NT * W)
                nc.scalar.dma_start(ydst, out_sb[:])

            # pipeline: dma(it) / replicate+convert(it-1) / compute(it-2)
            D = 2
            pipe = [None] * D
            rep_ctx = (tc.For_i(0, reps, 1,
                                hint_engines=(mybir.EngineType.PE,
                                              mybir.EngineType.SP,
                                              mybir.EngineType.Activation,
                                              mybir.EngineType.DVE))
                       if reps > 1 else nullcontext())
            with rep_ctx:
                for it in range(N_ITERS + D):
                    if it < N_ITERS:
                        st0 = dma_in(it)
                    if 1 <= it < N_ITERS + 1:
                        pipe[(it - 1) % D] = replicate_convert(
                            pipe[(it - 1) % D])
                    if it >= D:
                        compute(pipe[it % D], it - D)
                    if it < N_ITERS:
                        pipe[it % D] = st0

    nc.compile()
    return nc


def _get_prog(reps=1):
    with _prog_lock:
        if reps not in _progs:
            _progs[reps] = _build_program(reps)
    return _progs[reps]


def _prep_inputs(features, depth, weight):
    f = np.ascontiguousarray(features, dtype=np.float32)
    d = np.ascontiguousarray(depth, dtype=np.int32)
    w = np.ascontiguousarray(weight, dtype=np.float32)

    fb = f.astype(bf16)  # (B, iC, H, W), no spatial padding
    dpad = np.zeros((B, H + 2, W + 2), dtype=np.int32)
    dpad[:, 1:-1, 1:-1] = d

    lay = _part_layout()

    # bmask[b, p, h, w] for partition layout `lay`; host zeroes the w-edge
    # columns of dw=+-1 taps (their flat-window reads wrap rows).
    bmask = np.empty((B, 128, H, W), dtype=ml_dtypes.float8_e4m3)
    bj_cache = {}
    for tap in TAP_B + TAP_Am + TAP_Ap:
        dh, dw = tap
        dj = dpad[:, 1 + dh:H + 1 + dh, 1 + dw:W + 1 + dw] - d
        bj = ((dj == 0).astype(np.float32)
              - (dj == -1).astype(np.float32))
        if dw == -1:
            bj[:, :, 0] = 0.0
        elif dw == 1:
            bj[:, :, W - 1] = 0.0
        bj_cache[tap] = bj.astype(ml_dtypes.float8_e4m3)
    for p, (tap, ch) in enumerate(lay):
        bmask[:, p] = bj_cache[tap]

    # weights: pA pass uses (W1+W0)/2, pB pass uses (W1-W0)/2, center W1
    wA = np.zeros((128, oC), np.float32)
    wB = np.zeros((128, oC), np.float32)
    for p, ((dh, dw), ch) in enumerate(lay):
        kh, kw = dh + 1, dw + 1
        w1 = w[:, ch, 1, kh, kw]
        w0 = w[:, ch, 0, kh, kw]
        wA[p, :] = 0.5 * (w1 + w0)
        wB[p, :] = 0.5 * (w1 - w0)
    wC = np.ascontiguousarray(w[:, :, 1, 1, 1].T)
    wA = wA.astype(bf16)
    wB = wB.astype(bf16)
    wC = wC.astype(bf16)

    in_maps = []
    for c in range(NCORES):
        b, r = c // 2, c % 2
        rows = slice(r * HC, (r + 1) * HC)
        # slab: rows (r*HC - 1) .. ((r+1)*HC), flat, with 1 leading pad elem
        xs = np.zeros((iC, (HC + 2) * W + PADW), dtype=bf16)
        top = r * HC - 1
        bot = (r + 1) * HC  # inclusive extra row
        rows_ext = np.zeros((iC, HC + 2, W), dtype=bf16)
        rows_ext[:, 1:HC + 1] = fb[b, :, rows, :]
        if top >= 0:
            rows_ext[:, 0] = fb[b, :, top, :]
        if bot < H:
            rows_ext[:, HC + 1] = fb[b, :, bot, :]
        xs[:, 1:1 + (HC + 2) * W] = rows_ext.reshape(iC, (HC + 2) * W)
        in_maps.append({
            "xs": xs,
            "bm": np.ascontiguousarray(
                bmask[b, :, rows, :]).reshape(128, HC * W),
            "wA": wA, "wB": wB, "wC": wC,
        })
    return in_maps


def _run(in_maps, trace=False, reps=1):
    from concourse.bass_utils import run_bass_kernel_spmd
    prog = _get_prog(reps)
    return run_bass_kernel_spmd(prog, in_maps, list(range(NCORES)),
                                trace=trace)


def kernel(features, depth, weight, _trace=False, _ret_raw=False):
    in_maps = _prep_inputs(features, depth, weight)
    res = _run(in_maps, trace=_trace)
    out = np.empty((B, oC, H, W), dtype=np.float32)
    for c in range(NCORES):
        b, r = c // 2, c % 2
        # y[it, g, o, t, w] -> rows h = R*it + 4*t + g
        yp = res.results[c]["y"].transpose(2, 0, 3, 1, 4)  # (o, it, t, g, w)
        out[b, :, r * HC:(r + 1) * HC, :] = \
            yp.reshape(oC, HC, W).astype(np.float32)
    if _ret_raw:
        return out, res
    return out


# revision 27
# speedup vs baseline: 2.3135x; 2.3135x over previous
"""Depth-gated 3x3 conv (DepConv3D) Trainium2 Bass kernel.

Shapes (hardcoded): features (4,16,512,512) f32, depth (4,512,512) int32,
weight (32,16,3,3,3) f32 -> out (4,32,512,512) f32.

Strategy: 8-way data parallel over (batch, row-half). Each core computes a
(32, 256, 512) output slab.

Math: for output pixel p and tap k (3x3 neighborhood), the weight depth-slice
is selected by diff = depth[nb_k(p)] - depth[p]: diff==0 -> W[:,:,1,k],
diff==-1 -> W[:,:,0,k], else no contribution. The center tap always uses
W[:,:,1,center]. With bmask = (diff==0)-(diff==-1) in {0,+1,-1} and
amask = bmask^2, the gated weight is amask*(W1+W0)/2 + bmask*(W1-W0)/2.

Host prep (layout only): bf16-cast features; build the x8 channel-replicated
shifted feature array x_rep[16j+i, h, w] = x[i, nb_j(h,w)] and the x16
channel-replicated signed gate planes bmask in {0,+1,-1} as fp8.

Per-core pipeline, per 8-row iteration (NF=4096 pixels):
  - DMA x_rep (128,NF+2) bf16 + bmask (128,NF) fp8 (sync ring).
  - ACT: convert bmask fp8 -> bf16.
  - DVE: pB = bmask*x_rep, pA = bmask*pB  (bmask^2 = amask, so only 2
    tensor_tensor ops, both 2x mode).
  - PE: per psum tile (4 col-tiled 32-out groups, tile_position=(0,32g)):
    psum = wB.T@pB + wC.T@xc + wA.T@pA, where xc (raw center pixel) is
    read as a +1-shifted window of x_rep tap group 0 (= shift (0,-1),
    stored FLAT so the window is exact at w=511 too).
  - ACT evicts both psum tiles -> one (128,1024) bf16 staging tile,
    one batched DMA to HBM (scalar ring).
Emission is software-pipelined (DMA k+2 / convert k+1 / compute k) so each
engine's queue order matches dataflow readiness.
"""

import sys
import threading

sys.path.insert(0, "/opt/trn_rl_repo")

import os
import numpy as np
import ml_dtypes

PROBE_NO_C = os.environ.get("PROBE_NO_C", "0") == "1"  # timing-only probe

bf16 = ml_dtypes.bfloat16

B, iC, H, W = 4, 16, 512, 512
oC = 32
NCORES = 8
HC = H // 2  # rows per core (256)
R = 8        # rows per iteration
NT = R // 4  # psum tiles per iteration
NF = R * W   # free elements per iteration
N_ITERS = HC // R
PADW = 64    # host pad elems after HC*W so the +2-window read stays in-bounds
# tap 0 must be (0,-1): its x_rep group doubles as the center-pixel source
# via a +1 window shift (x[i,h,w] = x_rep[0:16, h, w+1]).
TAPS = [(0, -1), (-1, -1), (-1, 0), (-1, 1), (0, 1), (1, -1), (1, 0), (1, 1)]

_prog_lock = threading.Lock()
_progs = {}


def _win_ap(base_ap, dims, offset_elems):
    """Hand-build an AP: dims = [(stride, size), ...] over base tensor."""
    ap = base_ap.copy()
    while ap.ndim > 1:
        ap = ap.flatten()
    ap = ap[offset_elems:offset_elems + 1]
    for _ in range(len(dims) - 1):
        ap = ap.unsqueeze(0)
    a = ap.ap
    for i, (st, sz) in enumerate(dims):
        a[i] = [st, sz]
    return ap


def _build_program(reps=1):
    import concourse.tile as tile
    from concourse import bacc, mybir
    from contextlib import ExitStack, nullcontext

    nc = bacc.Bacc("TRN2", target_bir_lowering=False, debug=False,
                   num_devices=NCORES)
    xrep_d = nc.dram_tensor("xrep", [128, HC * W + PADW], mybir.dt.bfloat16,
                            kind="ExternalInput").ap()
    bm_d = nc.dram_tensor("bm", [128, HC * W], mybir.dt.float8e4,
                          kind="ExternalInput").ap()
    wA = nc.dram_tensor("wA", [128, oC], mybir.dt.bfloat16,
                        kind="ExternalInput").ap()
    wB = nc.dram_tensor("wB", [128, oC], mybir.dt.bfloat16,
                        kind="ExternalInput").ap()
    wC = nc.dram_tensor("wC", [iC, oC], mybir.dt.bfloat16,
                        kind="ExternalInput").ap()
    y = nc.dram_tensor("y", [HC // R, 4, oC, NT, W], mybir.dt.bfloat16,
                       kind="ExternalOutput").ap()

    with tile.TileContext(nc) as tc:
        with ExitStack() as ctx:
            wpool = ctx.enter_context(tc.tile_pool(name="w", bufs=1))
            inpool = ctx.enter_context(tc.tile_pool(name="in", bufs=5))
            mpool = ctx.enter_context(tc.tile_pool(name="m", bufs=3))
            opool = ctx.enter_context(tc.tile_pool(name="o", bufs=3))
            pspool = ctx.enter_context(
                tc.tile_pool(name="ps", bufs=8, space="PSUM"))

            wA_t = wpool.tile([128, oC], mybir.dt.bfloat16, tag="wA")
            wB_t = wpool.tile([128, oC], mybir.dt.bfloat16, tag="wB")
            wC_t = wpool.tile([iC, oC], mybir.dt.bfloat16, tag="wC")
            nc.sync.dma_start(wA_t[:], wA[:])
            nc.sync.dma_start(wB_t[:], wB[:])
            nc.sync.dma_start(wC_t[:], wC[:])

            def dma_in(it):
                h0 = it * R
                # NF+2 elems: +1 for the center window, +1 so the per-
                # partition transfer is 8196B (4B aligned)
                x_rep = inpool.tile([128, NF + 2], mybir.dt.bfloat16,
                                    tag="xrep")
                src = _win_ap(xrep_d, [(HC * W + PADW, 128), (1, NF + 2)],
                              h0 * W)
                nc.sync.dma_start(x_rep[:], src)
                bm8 = inpool.tile([128, NF], mybir.dt.float8e4, tag="bm8")
                bsrc = _win_ap(bm_d, [(HC * W, 128), (1, NF)], h0 * W)
                nc.sync.dma_start(bm8[:], bsrc)
                return x_rep, bm8

            def convert(st):
                x_rep, bm8 = st
                bm = mpool.tile([128, NF], mybir.dt.bfloat16, tag="bm")
                nc.scalar.copy(bm[:], bm8[:])
                return x_rep, bm

            def compute(st, it):
                x_rep, bm = st
                pB = mpool.tile([128, NF], mybir.dt.bfloat16, tag="pB")
                pA = mpool.tile([128, NF], mybir.dt.bfloat16, tag="pA")
                nc.vector.tensor_tensor(pB[:], bm[:], x_rep[:, 0:NF],
                                        mybir.AluOpType.mult)
                nc.vector.tensor_tensor(pA[:], bm[:], pB[:],
                                        mybir.AluOpType.mult)

                out_sb = opool.tile([128, NT * W], mybir.dt.bfloat16,
                                    tag="osb")
                for t in range(NT):  # psum tiles: rows h0+4t .. h0+4t+3
                    psum = pspool.tile([128, W], mybir.dt.float32,
                                       tag="psum")
                    # pass-major issue: 4 col-tiled groups (distinct
                    # col_grp). B first (ready earliest), C (raw x_rep),
                    # A last (depends on pB).
                    passes = ((wB_t, pB, 128, 0, True, False),
                              (wC_t, x_rep, iC, 1, False, False),
                              (wA_t, pA, 128, 0, False, True))
                    if PROBE_NO_C:
                        passes = ((wB_t, pB, 128, 0, True, False),
                                  (wA_t, pA, 128, 0, False, True))
                    for lhsT, rhs, np_, off, start, stop in passes:
                        for g in range(4):
                            r = 4 * t + g
                            sl = slice(r * W + off, (r + 1) * W + off)
                            nc.tensor.matmul(psum[32 * g:32 * g + 32, :],
                                             lhsT[:], rhs[0:np_, sl],
                                             start=start, stop=stop,
                                             tile_position=(0, 32 * g),
                                             skip_group_check=True)
                    nc.scalar.copy(out_sb[:, t * W:(t + 1) * W], psum[:])

                # packed output: y[it, g, o, t, w] = out row (R*it+4t+g),
                # channel o = out_sb[32g+o, t*W+w] -> one dense DMA
                ydst = _win_ap(y, [(NT * W, 128), (1, NT * W)],
                               it * 128 * NT * W)
                nc.scalar.dma_start(ydst, out_sb[:])

            # software-pipelined emission: DMA it / convert it-1 / rest it-2
            D = 2
            pipe = [None] * D
            rep_ctx = (tc.For_i(0, reps, 1,
                                hint_engines=(mybir.EngineType.PE,
                                              mybir.EngineType.SP,
                                              mybir.EngineType.Activation,
                                              mybir.EngineType.DVE))
                       if reps > 1 else nullcontext())
            with rep_ctx:
                for it in range(N_ITERS + D):
                    if it < N_ITERS:
                        st0 = dma_in(it)
                    if 1 <= it < N_ITERS + 1:
                        pipe[(it - 1) % D] = convert(pipe[(it - 1) % D])
                    if it >= D:
                        compute(pipe[it % D], it - D)
                    if it < N_ITERS:
                        pipe[it % D] = st0

    nc.compile()
    return nc


def _get_prog(reps=1):
    with _prog_lock:
        if reps not in _progs:
            _progs[reps] = _build_program(reps)
    return _progs[reps]


def _prep_inputs(features, depth, weight):
    f = np.ascontiguousarray(features, dtype=np.float32)
    d = np.ascontiguousarray(depth, dtype=np.int32)
    w = np.ascontiguousarray(weight, dtype=np.float32)

    fpad = np.zeros((B, iC, H + 2, W + 2), dtype=bf16)
    fpad[:, :, 1:-1, 1:-1] = f.astype(bf16)
    dpad = np.zeros((B, H + 2, W + 2), dtype=np.int32)
    dpad[:, 1:-1, 1:-1] = d

    # x_rep[b, 16j+i, h, w] = fpad[b, i, 1+h+dh_j, 1+w+dw_j]
    # bmask = (diff==0) - (diff==-1) in {0,+1,-1}
    x_rep = np.empty((B, 128, H, W), dtype=bf16)
    bmask = np.empty((B, 128, H, W), dtype=ml_dtypes.float8_e4m3)
    for j, (dh, dw) in enumerate(TAPS):
        if j == 0:
            # tap 0 = (0,-1) built as a FLAT shift-by-1 of the center
            # stream, so the kernel's +1-window read of this group yields
            # the exact center pixel everywhere (incl. w=511, where the
            # flat layout holds x[h,511] at position (h+1,0)). The one
            # position whose tap value this corrupts, w=0, is zeroed in
            # the mask below (reference contributes 0 there: the (0,-1)
            # neighbor of w=0 is zero padding).
            xf = f.astype(bf16).reshape(B, iC, H * W)
            t0 = np.zeros((B, iC, H * W), dtype=bf16)
            t0[:, :, 1:] = xf[:, :, :-1]
            x_rep[:, 0:16] = t0.reshape(B, iC, H, W)
        else:
            x_rep[:, 16 * j:16 * j + 16] = \
                fpad[:, :, 1 + dh:H + 1 + dh, 1 + dw:W + 1 + dw]
        dj = dpad[:, 1 + dh:H + 1 + dh, 1 + dw:W + 1 + dw] - d
        bj = ((dj == 0).astype(np.float32)
              - (dj == -1).astype(np.float32))
        if j == 0:
            bj[:, :, 0] = 0.0
        bmask[:, 16 * j:16 * j + 16] = \
            bj.astype(ml_dtypes.float8_e4m3)[:, None, :, :]

    # weight passes: pA uses Ws=(W1+W0)/2 (gate |b|); pB uses Wd=(W1-W0)/2
    # (gate b): |b|*Ws + b*Wd == m1*W1 + m0*W0
    wA = np.zeros((128, oC), np.float32)
    wB = np.zeros((128, oC), np.float32)
    for j, (dh, dw) in enumerate(TAPS):
        kh, kw = dh + 1, dw + 1
        w1 = w[:, :, 1, kh, kw].T
        w0 = w[:, :, 0, kh, kw].T
        wA[16 * j:16 * j + 16, :] = 0.5 * (w1 + w0)
        wB[16 * j:16 * j + 16, :] = 0.5 * (w1 - w0)
    wC = np.ascontiguousarray(w[:, :, 1, 1, 1].T)
    wA = wA.astype(bf16)
    wB = wB.astype(bf16)
    wC = wC.astype(bf16)

    in_maps = []
    for c in range(NCORES):
        b, r = c // 2, c % 2
        rows = slice(r * HC, (r + 1) * HC)
        xr = np.zeros((128, HC * W + PADW), dtype=bf16)
        xr[:, :HC * W] = x_rep[b, :, rows, :].reshape(128, HC * W)
        # the +1-window read of tap group 0 at the slab's last pixel lands
        # on pad element HC*W: it must hold the last center value
        xr[0:16, HC * W] = f[b].astype(bf16)[:, (r + 1) * HC - 1, W - 1]
        in_maps.append({
            "xrep": xr,
            "bm": np.ascontiguousarray(
                bmask[b, :, rows, :]).reshape(128, HC * W),
            "wA": wA, "wB": wB, "wC": wC,
        })
    return in_maps


def _run(in_maps, trace=False, reps=1):
    from concourse.bass_utils import run_bass_kernel_spmd
    prog = _get_prog(reps)
    return run_bass_kernel_spmd(prog, in_maps, list(range(NCORES)),
                                trace=trace)


def kernel(features, depth, weight, _trace=False, _ret_raw=False):
    in_maps = _prep_inputs(features, depth, weight)
    res = _run(in_maps, trace=_trace)
    out = np.empty((B, oC, H, W), dtype=np.float32)
    for c in range(NCORES):
        b, r = c // 2, c % 2
        # y[it, g, o, t, w] -> rows h = R*it + 4*t + g
        yp = res.results[c]["y"].transpose(2, 0, 3, 1, 4)  # (o, it, t, g, w)
        out[b, :, r * HC:(r + 1) * HC, :] = \
            yp.reshape(oC, HC, W).astype(np.float32)
    if _ret_raw:
        return out, res
    return out
